# revision 1
# baseline (speedup 1.0000x reference)
"""Trainium2 Bass kernel for nn_Attention_42348377538911.

3D attention: x [2, 128, 16, 16, 16] -> qkv 1x1x1 conv -> 4-head attention
over N=4096 positions (dim_head=32) -> out 1x1x1 conv.

Sharding: 8 cores = 2 batches x 4 heads (one (b, h) pair per core).
Each core computes its head's attention and a tensor-parallel partial of the
output projection (w_out split along hidden); host sums the 4 partials per
batch and adds b_out.

Per-core kernel layout (all attention math in simT = [j, i] orientation so no
transposes are ever needed):
  qk-proj : psum[64, 512] = w_qkT.T @ x-tile          (q rows scaled by d^-1/2)
  vT-proj : psum[128, 32] = x-chunk.T @ w_vT          (vT directly, x stationary)
  simT    : psum[128j, 512i] = k-chunk.T @ q-tile     (f32r, 4x row-packed K=32)
  exp     : ACT reads 4(3)-bank psum group, writes SBUF f32r
  AV+sums : psum[33, 512] += vT_aug-chunk.T @ expT    (col 32 of vT_aug = ones
                                                       -> row 32 = softmax denom)
  norm    : recip of sums (partition-0 hop) -> gpsimd partition_broadcast ->
            DVE multiply
  y-proj  : psum[128, 512] = w_oT.T @ out_normT; copy to SBUF; DMA out

PSUM budget: qkA 4 banks + qkB 3 banks (alternating exp groups, double
buffered against each other) + av 1 bank = 8.  y-proj borrows the qkB slot.
"""

import sys

import numpy as np

if "/opt/trn_rl_repo" not in sys.path:
    sys.path.insert(0, "/opt/trn_rl_repo")

HEADS = 4
DIM_HEAD = 32
B = 2
C = 128
N = 4096          # 16*16*16 spatial positions
NT = 512          # i-tile width
N_IT = N // NT    # 8 i-tiles
A_GROUPS = 5      # chunks 7g .. 7g+4   (4-wide, 4 psum banks)
B_GROUPS = 4      # chunks 7g+4 .. 7g+7 (3-wide, 3 psum banks)

_cached = {}


def _build(reps=1):
    import concourse.bacc as bacc
    import concourse.tile as tile
    import concourse.mybir as mybir
    from concourse.bass import ts

    f32 = mybir.dt.float32
    f32r = mybir.dt.float32r
    EXP = mybir.ActivationFunctionType.Exp

    nc = bacc.Bacc("TRN2", target_bir_lowering=False, debug=False, num_devices=8)
    x_d = nc.dram_tensor("x", [C, N], f32, kind="ExternalInput").ap()
    w4q_d = nc.dram_tensor("w_4q", [C, C], f32, kind="ExternalInput").ap()
    w4k_d = nc.dram_tensor("w_4k", [C, C], f32, kind="ExternalInput").ap()
    wvt_d = nc.dram_tensor("w_vT", [C, DIM_HEAD], f32, kind="ExternalInput").ap()
    wot_d = nc.dram_tensor("w_oT", [DIM_HEAD, C], f32, kind="ExternalInput").ap()
    y_d = nc.dram_tensor("y", [C, N], f32, kind="ExternalOutput").ap()

    # processing order per i-tile: A0 B0 A1 B1 A2 B2 A3 B3 A4
    seq = []
    for g in range(A_GROUPS):
        seq.append(("A", g, 7 * g, 4))
        if g < B_GROUPS:
            seq.append(("B", g, 7 * g + 4, 3))

    with tile.TileContext(nc) as tc:
        with tc.tile_pool(name="sing", bufs=1) as sing:
            # long-lived SBUF tensors
            w4q = sing.tile([C, C], f32r)
            w4k = sing.tile([C, C], f32r)
            wvt = sing.tile([C, DIM_HEAD], f32r)
            wot = sing.tile([DIM_HEAD, C], f32r)
            x_sb = [sing.tile([C, NT], f32r, tag=f"x{cx}", name=f"x{cx}")
                    for cx in range(8)]
            # q replicated at 4 partition bases, one tile per i-tile and
            # one k tile per group so QK deps are tile-granular
            q_rt = [sing.tile([128, NT], f32r, tag=f"qrt{it}", name=f"qrt{it}")
                    for it in range(N_IT)]
            k_rt = [sing.tile([128, 128], f32r, tag=f"krt{kg}", name=f"krt{kg}")
                    for kg in range(9)]
            vt_aug = sing.tile([128, 32, 33], f32r)     # per chunk [j, d + ones]
            scr = sing.tile([1, 64], f32)

            nc.sync.dma_start(w4q, w4q_d.bitcast(f32r))
            nc.sync.dma_start(w4k, w4k_d.bitcast(f32r))
            nc.sync.dma_start(wvt, wvt_d.bitcast(f32r))
            nc.sync.dma_start(wot, wot_d.bitcast(f32r))
            # x in 8 chunks so the first projection starts after 512 cols;
            # issue these before the exp-table warm-up so its ~2.7us
            # ACT_TABLE_LOAD doesn't block the x0 issue on the scalar queue
            for cx in range(8):
                nc.scalar.dma_start(x_sb[cx], x_d[:, ts(cx, NT)].bitcast(f32r))
            # warm the ACT exp table set while P0 runs
            nc.vector.memset(scr, 0.0)
            nc.scalar.activation(scr, scr, EXP)
            nc.vector.memset(vt_aug[:].bitcast(f32), 1.0)

            for rep in range(reps):
                # ------- P0: replicated projections (no layout DMAs) -------
                # chunk jc -> (group index, row) in processing layout
                chunk_pos = {}
                for kind, g, jc0, width in seq:
                    kg = g if kind == "A" else A_GROUPS + g
                    for r in range(width):
                        chunk_pos[jc0 + r] = (kg, r)

                with tc.tile_pool(name="p0ps", bufs=2, space="PSUM") as p0ps:
                    if rep == 0:
                        warm = p0ps.tile([128, NT], f32, tag="pq")
                        for _ in range(7):
                            nc.tensor.matmul(warm[:, 0:C], lhsT=w4q, rhs=w4q,
                                             start=True, stop=True)
                    for it in range(N_IT):
                        xs = x_sb[it]
                        psq = p0ps.tile([128, NT], f32, tag="pq")
                        nc.tensor.matmul(psq, lhsT=w4q, rhs=xs,
                                         start=True, stop=True)
                        nc.vector.tensor_copy(q_rt[it], psq)
                        psk = p0ps.tile([128, NT], f32, tag="pk")
                        nc.tensor.matmul(psk, lhsT=w4k, rhs=xs,
                                         start=True, stop=True)
                        for jj in range(4):
                            kg, r = chunk_pos[4 * it + jj]
                            nc.vector.tensor_copy(
                                k_rt[kg][32 * r:32 * r + 32, :],
                                psk[32 * r:32 * r + 32, ts(jj, 128)])

                # ---------------- P1: attention ----------------
                with tc.tile_pool(name="exA", bufs=10) as exA_pool, \
                     tc.tile_pool(name="exB", bufs=9) as exB_pool, \
                     tc.tile_pool(name="nrm", bufs=2) as nrm, \
                     tc.tile_pool(name="ysb", bufs=2) as ysb, \
                     tc.tile_pool(name="qkA", bufs=1, space="PSUM") as qkA, \
                     tc.tile_pool(name="qkB", bufs=1, space="PSUM") as qkB, \
                     tc.tile_pool(name="avp", bufs=1, space="PSUM") as avp:

                    ex_tiles = [None] * (N_IT + 1)
                    for step in range(N_IT + 1):
                        # QK + exp for i-tile `step`
                        if step < N_IT:
                            cur = []
                            for kind, g, jc0, width in seq:
                                if kind == "A":
                                    qk_ps = qkA.tile([128, 4 * NT], f32,
                                                     tag="qkA")
                                    ex_t = exA_pool.tile([128, 4 * NT], f32r,
                                                         tag="exA")
                                    kg = g
                                else:
                                    qk_ps = qkB.tile([128, 3 * NT], f32,
                                                     tag="qkB")
                                    ex_t = exB_pool.tile([128, 3 * NT], f32r,
                                                         tag="exB")
                                    kg = A_GROUPS + g
                                for r in range(width):
                                    nc.tensor.matmul(
                                        qk_ps[:, ts(r, NT)],
                                        lhsT=k_rt[kg][32 * r:32 * r + 32, :],
                                        rhs=q_rt[step][32 * r:32 * r + 32, :],
                                        start=True, stop=True,
                                        tile_position=(32 * r, 0))
                                nc.scalar.activation(ex_t, qk_ps, EXP)
                                cur.append((ex_t, jc0, width))
                            ex_tiles[step] = cur

                        # vT projection during step 0, borrowing the av bank
                        if step == 0:
                            for half in range(2):
                                ps2 = avp.tile([128, 512], f32, tag="avy")
                                for jj in range(16):
                                    jc = half * 16 + jj
                                    nc.tensor.matmul(
                                        ps2[:, ts(jj, 32)],
                                        lhsT=x_sb[jc // 4][:, ts(jc % 4, 128)],
                                        rhs=wvt,
                                        start=True, stop=True)
                                nc.vector.tensor_copy(
                                    vt_aug[:, half * 16:(half + 1) * 16,
                                           0:DIM_HEAD],
                                    ps2[:].rearrange("p (c d) -> p c d",
                                                     d=DIM_HEAD))

                        # AV + normalize + y for i-tile `step - 1`
                        if step > 0:
                            it = step - 1
                            av_ps = avp.tile([33, NT], f32, tag="avy")
                            n_mm = 0
                            for ex_t, jc0, width in ex_tiles[it]:
                                for r in range(width):
                                    nc.tensor.matmul(
                                        av_ps,
                                        lhsT=vt_aug[:, jc0 + r, :],
                                        rhs=ex_t[:, ts(r, NT)],
                                        start=(n_mm == 0), stop=(n_mm == 31))
                                    n_mm += 1
                            ex_tiles[it] = None

                            # single copy evacuates av (data + sums row); the
                            # av bank frees for the next i-tile immediately
                            on_raw = nrm.tile([33, NT], f32r, tag="onr")
                            nc.vector.tensor_copy(on_raw, av_ps)

                            # recip chain (concurrent with y matmul):
                            # sums row -> partition 0 -> recip -> bcast x128
                            t_s0 = nrm.tile([1, NT], f32, tag="ts0")
                            nc.sync.dma_start(t_s0, on_raw[32:33, :].bitcast(f32))
                            t_rc = nrm.tile([1, NT], f32, tag="trc")
                            nc.vector.reciprocal(t_rc, t_s0)
                            t_rcb = nrm.tile([128, NT], f32, tag="trcb")
                            nc.gpsimd.partition_broadcast(t_rcb, t_rc,
                                                          channels=128)

                            # y projection on unnormalized rows (linear in the
                            # per-column scale), borrows the qkB psum slot
                            y_ps = qkB.tile([128, NT], f32, tag="qkB")
                            nc.tensor.matmul(y_ps, lhsT=wot,
                                             rhs=on_raw[0:32, :],
                                             start=True, stop=True)
                            y_sb = ysb.tile([128, NT], f32, tag="ysb")
                            nc.vector.tensor_mul(y_sb, y_ps, t_rcb)
                            nc.sync.dma_start(y_d[:, ts(it, NT)], y_sb)

    nc.compile()
    return nc


def _get_nc():
    if "nc" not in _cached:
        _cached["nc"] = _build()
    return _cached["nc"]


def _make_in_maps(x, w_qkv, w_out):
    scale = DIM_HEAD ** -0.5
    in_maps = []
    for core in range(8):
        b, h = core // HEADS, core % HEADS
        w_q = w_qkv[h * DIM_HEAD:(h + 1) * DIM_HEAD, :]
        w_k = w_qkv[128 + h * DIM_HEAD:128 + (h + 1) * DIM_HEAD, :]
        w_v = w_qkv[256 + h * DIM_HEAD:256 + (h + 1) * DIM_HEAD, :]
        in_maps.append({
            "x": np.ascontiguousarray(x[b].reshape(C, N)),
            "w_4q": np.ascontiguousarray(np.tile(w_q.T * scale, (1, 4))),
            "w_4k": np.ascontiguousarray(np.tile(w_k.T, (1, 4))),
            "w_vT": np.ascontiguousarray(w_v.T),
            "w_oT": np.ascontiguousarray(
            w_out[:, h * DIM_HEAD:(h + 1) * DIM_HEAD].T),
        })
    return in_maps


def _gather(results, b_out):
    y = np.zeros((B, C, N), dtype=np.float32)
    for core in range(8):
        y[core // HEADS] += results[core]["y"]
    y += b_out.astype(np.float32)[None, :, None]
    return y.reshape(B, C, 16, 16, 16)


def run(x, w_qkv, w_out, b_out, trace=False):
    from concourse.bass_utils import run_bass_kernel_spmd
    nc = _get_nc()
    in_maps = _make_in_maps(np.asarray(x), np.asarray(w_qkv), np.asarray(w_out))
    res = run_bass_kernel_spmd(nc, in_maps, core_ids=list(range(8)),
                           trace=trace)
    return _gather(res.results, np.asarray(b_out)), res


def kernel(x, w_qkv, w_out, b_out):
    y, _ = run(x, w_qkv, w_out, b_out)
    return y



# revision 13
# speedup vs baseline: 1.4298x; 1.4298x over previous
"""Trainium2 Bass kernel for nn_Attention_42348377538911.

3D attention: x [2, 128, 16, 16, 16] -> qkv 1x1x1 conv -> 4-head attention
over N=4096 positions (dim_head=32) -> out 1x1x1 conv.

Sharding: 8 cores = 2 batches x 4 heads (one (b, h) pair per core).
Each core computes its head's attention and a tensor-parallel partial of the
output projection; host sums the 4 partials per batch and adds b_out.

Cost-model-driven layout (per core):
  P0    : psum[64, 1024] = w_qkT.T @ x-bf16 (q rows scaled); ACT/DVE evacuate
          q/k to f32r SBUF.  vT = x-chunk.T @ w_v into one [128, 1024] psum,
          single-instruction evacuation into ones-augmented vt_aug bf16.
  simT  : psum[128j, 512i] = k-chunk.T @ q  (f32r, 1 cyc/row), two j-chunks
          per [128, 1024] psum tile.
  exp   : split between ACT (true exp -> bf16) and DVE (Schraudolph:
          i16 = rint(s*128/ln2 + 16250.5), bits reinterpreted as bf16).
  AV    : flipped orientation: psum[128i, 33] += ex-tile.T @ vt_aug
          (bf16, 33 cyc/matmul; col 32 of vt_aug = ones -> softmax denom).
  norm  : DVE reciprocal of denom + broadcast multiply -> out_norm bf16.
  outT  : DMA-transpose [128, 128] (i x (c d) -> (c d) x i).
  y     : psum[128, 128c] = w_oT-rep.T @ outT-chunk; ACT/DVE evacuate to
          bf16; DMA to DRAM.
"""

import sys

import numpy as np

if "/opt/trn_rl_repo" not in sys.path:
    sys.path.insert(0, "/opt/trn_rl_repo")

HEADS = 4
DIM_HEAD = 32
B = 2
C = 128
N = 4096          # 16*16*16 spatial positions
NT = 512          # i-tile width
N_IT = N // NT    # 8 i-tiles

A7 = 128.0 / np.log(2.0)    # Schraudolph scale for bf16 bit pattern
B7 = 16250.5                # calibrated offset (min max-rel-err)

_cached = {}


def _build(nd=8, dma_tr=False, inter_av=True, schraud=True, pool_ms=True):
    import concourse.bacc as bacc
    import concourse.tile as tile
    import concourse.mybir as mybir
    from concourse.bass import ts
    from concourse import masks

    f32 = mybir.dt.float32
    f32r = mybir.dt.float32r
    bf16 = mybir.dt.bfloat16
    i16 = mybir.dt.int16
    EXP = mybir.ActivationFunctionType.Exp
    ALU = mybir.AluOpType

    nc = bacc.Bacc("TRN2", target_bir_lowering=False, debug=False, num_devices=nd)
    x_d = nc.dram_tensor("x", [C, N], bf16, kind="ExternalInput").ap()
    wqk_d = nc.dram_tensor("w_qk", [C, 64], bf16, kind="ExternalInput").ap()
    wvt_d = nc.dram_tensor("w_vT", [C, DIM_HEAD], bf16, kind="ExternalInput").ap()
    wor_d = nc.dram_tensor("w_oR", [C, C], bf16, kind="ExternalInput").ap()
    y_d = nc.dram_tensor("y", [C, N], bf16, kind="ExternalOutput").ap()

    with tile.TileContext(nc) as tc:
        with tc.tile_pool(name="sing", bufs=1) as sing, \
             tc.tile_pool(name="exp", bufs=22) as expp, \
             tc.tile_pool(name="onrm", bufs=2) as onrm, \
             tc.tile_pool(name="odt", bufs=2) as odt, \
             tc.tile_pool(name="ysb", bufs=2) as ysbp, \
             tc.tile_pool(name="rcp", bufs=2) as rcpp:
            wqk = sing.tile([C, 64], bf16)
            wvt = sing.tile([C, DIM_HEAD], bf16)
            wor = sing.tile([C, C], bf16)
            x_sb = [sing.tile([C, 1024], bf16, tag=f"x{cx}", name=f"x{cx}")
                    for cx in range(4)]
            # q/k in f32r: q_t/k_t[i2] hold columns 1024*i2 .. +1024
            q_t = [sing.tile([32, 1024], f32r, tag=f"q{cx}", name=f"q{cx}")
                   for cx in range(4)]
            k_t = [sing.tile([32, 1024], f32r, tag=f"k{cx}", name=f"k{cx}")
                   for cx in range(4)]
            vt_aug = sing.tile([C, 32, 33], bf16)   # [j, chunk, d + ones]

            nc.sync.dma_start(wqk, wqk_d)
            nc.sync.dma_start(wvt, wvt_d)
            nc.sync.dma_start(wor, wor_d)
            for cx in range(4):
                nc.scalar.dma_start(x_sb[cx], x_d[:, 1024 * cx:1024 * (cx + 1)])
            # ones for the denominator column (overwritten cols get vT)
            if pool_ms:
                nc.gpsimd.memset(vt_aug[:], 1.0)
            else:
                nc.vector.memset(vt_aug[:], 1.0)
            ident = None
            if not dma_tr:
                ident = sing.tile([C, C], bf16)
                masks.make_identity(nc, ident[:])
            # warm the ACT exp table while DMAs run
            scr = sing.tile([1, 64], f32)
            nc.vector.memset(scr, 0.0)
            nc.scalar.activation(scr, scr, EXP)

            with tc.tile_pool(name="p0qk", bufs=2, space="PSUM") as p0qk, \
                 tc.tile_pool(name="p0v", bufs=1, space="PSUM") as p0v:

                # ---- PE warm-up: ~25 junk matmuls on the weight tiles ----
                warm = p0v.tile([C, 1024], f32, tag="pv")
                for _ in range(25):
                    nc.tensor.matmul(warm[:, 0:128], lhsT=wor, rhs=wor,
                                     start=True, stop=True)

                # ---------------- P0: projections ----------------
                for i2 in range(4):
                    pqk = p0qk.tile([64, 1024], f32, tag="pqk")
                    for h in range(2):
                        nc.tensor.matmul(pqk[:, ts(h, NT)], lhsT=wqk,
                                         rhs=x_sb[i2][:, ts(h, NT)],
                                         start=True, stop=True)
                    # evacuate q (ACT) and k (DVE) to f32r
                    nc.scalar.copy(q_t[i2][:], pqk[0:32, :].bitcast(f32r))
                    nc.vector.tensor_copy(k_t[i2][:], pqk[32:64, :].bitcast(f32r))

                # vT: one [128, 1024] psum holds all 32 chunks x 32 dims
                pv = p0v.tile([C, 1024], f32, tag="pv")
                for jc in range(32):
                    nc.tensor.matmul(
                        pv[:, ts(jc, 32)],
                        lhsT=x_sb[jc // 8][:, ts(jc % 8, 128)],
                        rhs=wvt, start=True, stop=True)
                nc.scalar.copy(
                    vt_aug[:, :, 0:DIM_HEAD],
                    pv[:].rearrange("p (c d) -> p c d", d=DIM_HEAD))

            with tc.tile_pool(name="simp", bufs=3, space="PSUM") as simp, \
                 tc.tile_pool(name="avp", bufs=1, space="PSUM") as avp, \
                 tc.tile_pool(name="yp", bufs=1, space="PSUM") as yp:

                # ---------------- P1: attention ----------------
                # per i-tile: 16 sim groups of [128, 1024] (2 j-chunks each)
                # exp engine pattern: 9 ACT / 7 DVE per 16 groups
                act_groups = {0, 2, 4, 6, 8, 10, 12, 14, 15}

                ex_tiles = [None] * N_IT
                for step in range(N_IT + 1):
                    do_sim = step < N_IT
                    do_av = step > 0
                    if do_av:
                        av = avp.tile([C, 132], f32, tag="av")
                        prev = ex_tiles[step - 1]
                    cur = []
                    for g in range(16):
                        if do_sim:
                            ps = simp.tile([C, 1024], f32, tag="sim")
                            for h in range(2):
                                jc = 2 * g + h
                                nc.tensor.matmul(
                                    ps[:, ts(h, NT)],
                                    lhsT=k_t[jc // 8][:, ts(jc % 8, 128)],
                                    rhs=q_t[step // 2][:, ts(step % 2, NT)],
                                    start=True, stop=True)
                            ex_t = expp.tile([C, 1024], bf16, tag="ex")
                            if g in act_groups or not schraud:
                                nc.scalar.activation(ex_t, ps, EXP)
                            else:
                                nc.vector.tensor_scalar(
                                    ex_t[:].bitcast(i16), ps, A7, B7,
                                    ALU.mult, ALU.add)
                            cur.append(ex_t)

                        # AV matmuls of the previous i-tile, interleaved.
                        # NOTE: each psum region's 32-step accumulation must
                        # run consecutively (c-major) — interleaving regions
                        # within a bank corrupts the accumulation (probe5).
                        if do_av and inter_av:
                            c = g // 4
                            for jj in range(8):
                                jc = 8 * (g % 4) + jj
                                ext = prev[jc // 2]
                                nc.tensor.matmul(
                                    av[:, 33 * c:33 * c + 33],
                                    lhsT=ext[:, ts(4 * (jc % 2) + c, 128)],
                                    rhs=vt_aug[:, jc, :],
                                    start=(jc == 0), stop=(jc == 31))
                    if do_av and not inter_av:
                        for c in range(4):
                            for jc in range(32):
                                ext = prev[jc // 2]
                                nc.tensor.matmul(
                                    av[:, 33 * c:33 * c + 33],
                                    lhsT=ext[:, ts(4 * (jc % 2) + c, 128)],
                                    rhs=vt_aug[:, jc, :],
                                    start=(jc == 0), stop=(jc == 31))
                    if do_sim:
                        ex_tiles[step] = cur

                    # tail of i-tile step-1: normalize, transpose, project
                    if do_av:
                        it = step - 1
                        av3 = av[:].rearrange("p (c w) -> p c w", w=33)
                        rcp = rcpp.tile([C, 4], f32, tag="rcp")
                        nc.vector.reciprocal(rcp, av3[:, :, 32])
                        onr = onrm.tile([C, C], bf16, tag="onr")
                        nc.vector.tensor_tensor(
                            onr[:].rearrange("p (c d) -> p c d", d=DIM_HEAD),
                            av3[:, :, 0:DIM_HEAD],
                            rcp[:, :, None].broadcast_to([C, 4, DIM_HEAD]),
                            ALU.mult)
                        if dma_tr:
                            dt_t = odt.tile([C, C], bf16, tag="dt")
                            nc.sync.dma_start(dt_t, onr, transpose=True)
                            ypt = yp.tile([C, NT], f32, tag="y")
                            for c in range(4):
                                nc.tensor.matmul(
                                    ypt[:, ts(c, 128)],
                                    lhsT=wor[32 * c:32 * c + 32, :],
                                    rhs=dt_t[32 * c:32 * c + 32, :],
                                    start=True, stop=True,
                                    tile_position=(32 * c, 0))
                        else:
                            ytr = yp.tile([C, NT], f32, tag="y")
                            trv = ytr[:].bitcast(bf16)
                            for c in range(4):
                                nc.tensor.transpose(
                                    trv[0:32, ts(c, 128)],
                                    onr[:, ts(c, 32)], ident[:])
                            dt_t = odt.tile([32, NT], bf16, tag="dt")
                            nc.scalar.copy(dt_t, trv[0:32, 0:NT])
                            ypt = yp.tile([C, NT], f32, tag="y")
                            nc.tensor.matmul(ypt, lhsT=wor[0:32, :],
                                             rhs=dt_t, start=True, stop=True)
                        y_sb = ysbp.tile([C, NT], bf16, tag="ysb")
                        if it % 2 == 0:
                            nc.scalar.copy(y_sb, ypt)
                        else:
                            nc.vector.tensor_copy(y_sb, ypt)
                        nc.sync.dma_start(y_d[:, ts(it, NT)], y_sb)
                        ex_tiles[it] = None

    nc.compile()
    return nc


def _get_nc():
    if "nc" not in _cached:
        _cached["nc"] = _build()
    return _cached["nc"]


def _to_bf16(a):
    import ml_dtypes
    return np.asarray(a, dtype=np.float32).astype(ml_dtypes.bfloat16)


def _make_in_maps(x, w_qkv, w_out):
    scale = DIM_HEAD ** -0.5
    in_maps = []
    for core in range(8):
        b, h = core // HEADS, core % HEADS
        w_q = w_qkv[h * DIM_HEAD:(h + 1) * DIM_HEAD, :]
        w_k = w_qkv[128 + h * DIM_HEAD:128 + (h + 1) * DIM_HEAD, :]
        w_v = w_qkv[256 + h * DIM_HEAD:256 + (h + 1) * DIM_HEAD, :]
        w_o = w_out[:, h * DIM_HEAD:(h + 1) * DIM_HEAD]  # [128, 32]
        in_maps.append({
            "x": _to_bf16(x[b].reshape(C, N)),
            "w_qk": _to_bf16(
                np.concatenate([w_q.T * scale, w_k.T], axis=1)),
            "w_vT": _to_bf16(w_v.T),
            "w_oR": _to_bf16(np.tile(w_o.T, (4, 1))),
        })
    return in_maps


def _gather(results, b_out):
    y = np.zeros((B, C, N), dtype=np.float32)
    for core in range(8):
        y[core // HEADS] += np.asarray(results[core]["y"], dtype=np.float32)
    y += b_out.astype(np.float32)[None, :, None]
    return y.reshape(B, C, 16, 16, 16)


def run(x, w_qkv, w_out, b_out, trace=False):
    from concourse.bass_utils import run_bass_kernel_spmd
    nc = _get_nc()
    in_maps = _make_in_maps(np.asarray(x), np.asarray(w_qkv), np.asarray(w_out))
    res = run_bass_kernel_spmd(nc, in_maps, core_ids=list(range(8)),
                               trace=trace)
    return _gather(res.results, np.asarray(b_out)), res


def kernel(x, w_qkv, w_out, b_out):
    y, _ = run(x, w_qkv, w_out, b_out)
    return y
